# revision 16
# baseline (speedup 1.0000x reference)
"""Haar-DWT downsampling + 1x1 conv + BN + ReLU fused Trainium2 kernel.

Math: the Haar DWT (J=1) followed by a 1x1 conv over the 4C subband
channels, inference BN, and ReLU folds into a 2x2/stride-2 conv:

    z[o, i, j] = relu( sum_{c,di,dj} Weff[o, c, di, dj] * x[c, 2i+di, 2j+dj]
                       + bias_total[o] )

with Weff/bias_total computed on the host from (W, b, gamma, beta, mean,
var).

Device mapping: SBUF partition = (dj, c) so each matmul contracts
K = 128 = (dj, c) in one shot with a full 128x128 weight (enables the
PE fast-weight-load path; K=64 variants measured ~3x slower per row).
Only 2 weight matrices (one per di tap); each 16-output-row PSUM tile
takes 8 matmuls (2 di x 4 banks).

Precision: x and weights bf16 on host, z produced in bf16 (PSUM
accumulation f32) — halves HBM traffic, which is the roofline.
Measured rel err ~3.5e-3 vs the 2e-2 gate.

DMA plan (SDMA read throughput measures ~13 B/ns per engine regardless
of descriptor size >= 8 KiB, so scheduling, not descriptor shaping, is
what's left):
- ALL x loads issue on the SP HWDGE ring in FIFO order; the two rings
  do NOT share engines fairly (ACT-ring work starves SP-ring work), so
  reads are never split across rings.  Weights/bias go on the ACT ring
  so they don't delay the first x chunk.
- One x tile holds the whole per-core input (partition (dj,c), free
  [bi, hh, pt, di, ilp, j] = 128 KiB/partition).  Loads are chunked
  (bi0,hh0,pt0) -> rest-of-bi0 -> bi1, per dj half, so the first
  matmuls start ~12 us in while the rest streams.
- x DRAM: [dj, g(16), cg(4), free (+2 KiB pad)]: DRAM AP
  [(g,16),(cg,4),(contig)] -> 16 outer entries, one per SDMA engine.
- z stores (bf16 tiles [hb, o, 2048], contiguous 512 KiB, outer dim
  o=128) issue on the GPSIMD SWDGE ring so they never block loads;
  host un-tiles.
- bias+ReLU alternates DVE / ACT so PSUM drain is two-engine wide.

Sharding: pure data-parallel over batch. B=16 -> 2 images per core on
8 cores.
"""

import numpy as np
import ml_dtypes

import concourse.bacc as bacc
import concourse.mybir as mybir
from concourse.tile import TileContext
from concourse.bass_utils import run_bass_kernel_spmd

BN_EPS = 1e-5

# Problem shape (hardcoded per harness contract)
B, C, H, W_IMG = 16, 64, 256, 256
COUT = 128
N_CORES = 8
B_LOCAL = B // N_CORES          # 2 images per core
HO, WO = H // 2, W_IMG // 2     # 128 x 128 output image

G = 16                          # channel groups (1 per SDMA engine)
CG = C // G                     # 4 channels per group
FREE1 = 2 * 4 * 2 * 16 * WO     # per-image free = [hh,pt,di,ilp,j] = 32768
FREE = B_LOCAL * FREE1          # per-partition free = 65536 elems (128 KiB)
PAD_E = 1024                    # 2 KiB bf16 pad per channel block
HB = 8                          # 16-output-row store blocks per image

F32 = mybir.dt.float32
BF16 = mybir.dt.bfloat16


def _fold_weights(W, b, gamma, beta, mean, var):
    """Fold DWT + conv + BN into per-di lhsT weights [2, 128(K=(dj,c)),
    128(M=o)] (bf16) and a per-channel f32 bias [COUT, 1]."""
    W = W.astype(np.float64)
    Wll, Wlh, Whl, Whh = W[:, :C], W[:, C:2 * C], W[:, 2 * C:3 * C], W[:, 3 * C:]
    s = (gamma.astype(np.float64) / np.sqrt(var.astype(np.float64) + BN_EPS))
    coef = {
        (0, 0): 0.5 * (Wll + Wlh + Whl + Whh),
        (0, 1): 0.5 * (Wll + Wlh - Whl - Whh),
        (1, 0): 0.5 * (Wll - Wlh + Whl - Whh),
        (1, 1): 0.5 * (Wll - Wlh - Whl + Whh),
    }
    bias_total = (b.astype(np.float64) * s + beta.astype(np.float64)
                  - mean.astype(np.float64) * s)
    lhsT = np.zeros((2, 128, COUT), dtype=np.float64)
    for di in range(2):
        for dj in range(2):
            wq = (coef[(di, dj)] * s[:, None]).T   # [c, o]
            lhsT[di, dj * C:(dj + 1) * C, :] = wq
    return (lhsT.astype(ml_dtypes.bfloat16),
            bias_total.astype(np.float32).reshape(COUT, 1))


def build_nc(run_bacc_compile=True):
    nc = bacc.Bacc(None)
    x = nc.dram_tensor("x", [2, G, CG, FREE + PAD_E], BF16,
                       kind="ExternalInput")
    w_lhsT = nc.dram_tensor("w_lhsT", [2, 128, COUT], BF16,
                            kind="ExternalInput")
    bias = nc.dram_tensor("bias", [COUT, 1], F32, kind="ExternalInput")
    # 32-output-row tiles: contiguous 1 MiB per store, 8 KiB descriptors
    z = nc.dram_tensor("z", [B_LOCAL, HB // 2, COUT, 32 * WO], BF16,
                       kind="ExternalOutput")

    relu = mybir.ActivationFunctionType.Relu

    with TileContext(nc) as tc:
        with (
            tc.tile_pool(name="consts", bufs=1) as cpool,
            tc.tile_pool(name="xin", bufs=1) as xpool,
            tc.tile_pool(name="psum", bufs=2, space="PSUM") as ppool,
            tc.tile_pool(name="zout", bufs=4) as zpool,
        ):
            # weights/bias on the ACT ring so they never delay x chunks
            w_sb = []
            for di in range(2):
                wt = cpool.tile([128, COUT], BF16, name=f"w{di}_sb")
                nc.scalar.dma_start(out=wt[:], in_=w_lhsT[di])
                w_sb.append(wt)
            bias_sb = cpool.tile([COUT, 1], F32)
            nc.scalar.dma_start(out=bias_sb[:], in_=bias[:])

            # whole per-core input in one tile: partition (dj, c), free
            # [bi, hh, pt, di, ilp, j] = 128 KiB bf16 per partition.
            # 8 chunks of 2 psum tiles each; dj=0 on the SP HWDGE ring,
            # dj=1 on the GPSIMD SWDGE ring — two concurrent read
            # streams per SDMA engine (the per-stream rate cap is ~13
            # B/ns; two streams overlap it).
            xt = xpool.tile([128, FREE], BF16)
            # Progressive chunk sizes: small first chunks (fast start,
            # issued entirely on the SP ring since the SWDGE Q7 takes
            # ~12 us to boot), then 32 KiB descriptors which amortize
            # best in the engines' queue rotation.
            bounds = (0, 4096, 8192, 16384, 32768, 49152, FREE)
            for k in range(len(bounds) - 1):
                lo, hi = bounds[k], bounds[k + 1]
                nc.sync.dma_start(
                    out=xt[0:C, lo:hi], in_=x[0, :, :, lo:hi])
                ring = nc.sync if k < 2 else nc.gpsimd
                ring.dma_start(
                    out=xt[C:2 * C, lo:hi], in_=x[1, :, :, lo:hi])
            xv = xt.rearrange(
                "p (bi hh pt di il j) -> p bi hh pt di il j",
                bi=B_LOCAL, hh=2, pt=4, di=2, j=WO,
            )
            for bi in range(B_LOCAL):
                for hh in range(2):
                    for pp in range(2):   # pt pair -> one 1 MiB store
                        zt = zpool.tile([COUT, 4096], BF16)
                        for sub in range(2):
                            pt = 2 * pp + sub
                            ps = ppool.tile([COUT, 2048], F32)
                            # output rows 64hh + 16pt + (4gg + 0..3)
                            for di in range(2):
                                for gg in range(4):
                                    nc.tensor.matmul(
                                        ps[:, gg * 512:gg * 512 + 512],
                                        lhsT=w_sb[di][:],
                                        rhs=xv[:, bi, hh, pt, di,
                                               4 * gg:4 * gg + 4, :],
                                        start=(di == 0),
                                        stop=(di == 1),
                                    )
                            zh = zt[:, sub * 2048:(sub + 1) * 2048]
                            # bias + ReLU; alternate engines so the PSUM
                            # drain is never single-engine-bound
                            if sub == 0:
                                nc.vector.tensor_scalar(
                                    zh, ps[:], bias_sb[:, 0:1], 0.0,
                                    mybir.AluOpType.add, mybir.AluOpType.max,
                                )
                            else:
                                nc.scalar.activation(
                                    zh, ps[:], relu, bias=bias_sb[:, 0:1],
                                )
                        # contiguous 1 MiB store, 8 KiB descriptors,
                        # DRAM outer dim o=128 -> sprays all engines.
                        # ACT ring: small, preempts reads briefly.
                        nc.scalar.dma_start(
                            out=z[bi, 2 * hh + pp], in_=zt[:])
    if run_bacc_compile:
        nc.compile()
    return nc


_NC_CACHE = {}


def _get_nc():
    if "nc" not in _NC_CACHE:
        _NC_CACHE["nc"] = build_nc()
    return _NC_CACHE["nc"]


def _tile_x(x):
    """[B, C, H, W] f32 -> [N_CORES, 2(dj), G, CG, FREE+PAD_E] bf16.

    Partition p = dj*64 + c (c = 4g+cg); per-partition free layout
    [bi, hh, pt, di, ilp, j] where input row = 128hh+32pt+2ilp+di,
    input col = 2j+dj.  2 KiB pad per channel block keeps the DMA AP
    dims from merging (preserves the 16x4 descriptor grouping).
    """
    xb = x.astype(ml_dtypes.bfloat16)
    # [core, bi, g, cg, hh, pt, ilp, di, j, dj]
    t = xb.reshape(N_CORES, B_LOCAL, G, CG, 2, 4, 16, 2, WO, 2)
    # -> [core, dj, g, cg, bi, hh, pt, di, ilp, j]
    t = np.ascontiguousarray(t.transpose(0, 9, 2, 3, 1, 4, 5, 7, 6, 8))
    t = t.reshape(N_CORES, 2, G, CG, FREE)
    out = np.zeros((N_CORES, 2, G, CG, FREE + PAD_E),
                   dtype=ml_dtypes.bfloat16)
    out[:, :, :, :, 0:FREE] = t
    return out


def kernel(x, W, b, gamma, beta, mean, var, _trace=False):
    x = np.asarray(x, dtype=np.float32)
    lhsT, bias_col = _fold_weights(
        np.asarray(W), np.asarray(b), np.asarray(gamma),
        np.asarray(beta), np.asarray(mean), np.asarray(var),
    )
    x_t = _tile_x(x)

    nc = _get_nc()
    in_maps = []
    for core in range(N_CORES):
        in_maps.append({"x": x_t[core], "w_lhsT": lhsT, "bias": bias_col})

    res = run_bass_kernel_spmd(
        nc, in_maps, list(range(N_CORES)), trace=_trace
    )
    # z tiles [b_local, hp, o, 32*WO] bf16 -> [B, COUT, HO, WO] f32
    zt = np.concatenate(
        [np.asarray(res.results[i]["z"]) for i in range(N_CORES)], axis=0
    ).astype(np.float32)
    out = np.ascontiguousarray(
        zt.reshape(B, HB // 2, COUT, 32, WO).transpose(0, 2, 1, 3, 4)
    ).reshape(B, COUT, HO, WO)
    if _trace:
        return out, res
    return out


# revision 17
# speedup vs baseline: 1.0707x; 1.0707x over previous
"""Haar-DWT downsampling + 1x1 conv + BN + ReLU fused Trainium2 kernel.

Math: the Haar DWT (J=1) followed by a 1x1 conv over the 4C subband
channels, inference BN, and ReLU folds into a 2x2/stride-2 conv:

    z[o, i, j] = relu( sum_{c,di,dj} Weff[o, c, di, dj] * x[c, 2i+di, 2j+dj]
                       + bias_total[o] )

with Weff/bias_total computed on the host from (W, b, gamma, beta, mean,
var).

Device mapping: SBUF partition = (dj, c) so each matmul contracts
K = 128 = (dj, c) in one shot with a full 128x128 weight (enables the
PE fast-weight-load path; K=64 variants measured ~3x slower per row).
Only 2 weight matrices (one per di tap); each 16-output-row PSUM tile
takes 8 matmuls (2 di x 4 banks).

Precision: x and weights bf16 on host, z produced in bf16 (PSUM
accumulation f32) — halves HBM traffic, which is the roofline.
Measured rel err ~3.5e-3 vs the 2e-2 gate.

DMA plan (SDMA read throughput measures ~13 B/ns per engine regardless
of descriptor size >= 8 KiB, so scheduling, not descriptor shaping, is
what's left):
- ALL x loads issue on the SP HWDGE ring in FIFO order; the two rings
  do NOT share engines fairly (ACT-ring work starves SP-ring work), so
  reads are never split across rings.  Weights/bias go on the ACT ring
  so they don't delay the first x chunk.
- One x tile holds the whole per-core input (partition (dj,c), free
  [bi, hh, pt, di, ilp, j] = 128 KiB/partition).  Loads are chunked
  (bi0,hh0,pt0) -> rest-of-bi0 -> bi1, per dj half, so the first
  matmuls start ~12 us in while the rest streams.
- x DRAM: [dj, g(16), cg(4), free (+2 KiB pad)]: DRAM AP
  [(g,16),(cg,4),(contig)] -> 16 outer entries, one per SDMA engine.
- z stores (bf16 tiles [hb, o, 2048], contiguous 512 KiB, outer dim
  o=128) issue on the GPSIMD SWDGE ring so they never block loads;
  host un-tiles.
- bias+ReLU alternates DVE / ACT so PSUM drain is two-engine wide.

Sharding: pure data-parallel over batch. B=16 -> 2 images per core on
8 cores.
"""

import numpy as np
import ml_dtypes

import concourse.bacc as bacc
import concourse.mybir as mybir
from concourse.tile import TileContext
from concourse.bass_utils import run_bass_kernel_spmd

BN_EPS = 1e-5

# Problem shape (hardcoded per harness contract)
B, C, H, W_IMG = 16, 64, 256, 256
COUT = 128
N_CORES = 8
B_LOCAL = B // N_CORES          # 2 images per core
HO, WO = H // 2, W_IMG // 2     # 128 x 128 output image

G = 16                          # channel groups (1 per SDMA engine)
CG = C // G                     # 4 channels per group
FREE1 = 2 * 4 * 2 * 16 * WO     # per-image free = [hh,pt,di,ilp,j] = 32768
FREE = B_LOCAL * FREE1          # per-partition free = 65536 elems (128 KiB)
PAD_E = 1024                    # 2 KiB bf16 pad per channel block
HB = 8                          # 16-output-row store blocks per image

F32 = mybir.dt.float32
BF16 = mybir.dt.bfloat16


def _fold_weights(W, b, gamma, beta, mean, var):
    """Fold DWT + conv + BN into per-di lhsT weights [2, 128(K=(dj,c)),
    128(M=o)] (bf16) and a per-channel f32 bias [COUT, 1]."""
    W = W.astype(np.float64)
    Wll, Wlh, Whl, Whh = W[:, :C], W[:, C:2 * C], W[:, 2 * C:3 * C], W[:, 3 * C:]
    s = (gamma.astype(np.float64) / np.sqrt(var.astype(np.float64) + BN_EPS))
    coef = {
        (0, 0): 0.5 * (Wll + Wlh + Whl + Whh),
        (0, 1): 0.5 * (Wll + Wlh - Whl - Whh),
        (1, 0): 0.5 * (Wll - Wlh + Whl - Whh),
        (1, 1): 0.5 * (Wll - Wlh - Whl + Whh),
    }
    bias_total = (b.astype(np.float64) * s + beta.astype(np.float64)
                  - mean.astype(np.float64) * s)
    lhsT = np.zeros((2, 128, COUT), dtype=np.float64)
    for di in range(2):
        for dj in range(2):
            wq = (coef[(di, dj)] * s[:, None]).T   # [c, o]
            lhsT[di, dj * C:(dj + 1) * C, :] = wq
    return (lhsT.astype(ml_dtypes.bfloat16),
            bias_total.astype(np.float32).reshape(COUT, 1))


def build_nc(run_bacc_compile=True):
    nc = bacc.Bacc(None)
    x = nc.dram_tensor("x", [2, G, CG, FREE + PAD_E], BF16,
                       kind="ExternalInput")
    w_lhsT = nc.dram_tensor("w_lhsT", [2, 128, COUT], BF16,
                            kind="ExternalInput")
    bias = nc.dram_tensor("bias", [COUT, 1], F32, kind="ExternalInput")
    # 32-output-row tiles: contiguous 1 MiB per store, 8 KiB descriptors
    z = nc.dram_tensor("z", [B_LOCAL, HB // 2, COUT, 32 * WO], BF16,
                       kind="ExternalOutput")

    relu = mybir.ActivationFunctionType.Relu

    with TileContext(nc) as tc:
        with (
            tc.tile_pool(name="consts", bufs=1) as cpool,
            tc.tile_pool(name="xin", bufs=1) as xpool,
            tc.tile_pool(name="psum", bufs=2, space="PSUM") as ppool,
            tc.tile_pool(name="zout", bufs=4) as zpool,
        ):
            # weights/bias on the ACT ring so they never delay x chunks
            w_sb = []
            for di in range(2):
                wt = cpool.tile([128, COUT], BF16, name=f"w{di}_sb")
                nc.scalar.dma_start(out=wt[:], in_=w_lhsT[di])
                w_sb.append(wt)
            bias_sb = cpool.tile([COUT, 1], F32)
            nc.scalar.dma_start(out=bias_sb[:], in_=bias[:])

            # whole per-core input in one tile: partition (dj, c), free
            # [bi, hh, pt, di, ilp, j] = 128 KiB bf16 per partition.
            # 8 chunks of 2 psum tiles each; dj=0 on the SP HWDGE ring,
            # dj=1 on the GPSIMD SWDGE ring — two concurrent read
            # streams per SDMA engine (the per-stream rate cap is ~13
            # B/ns; two streams overlap it).
            xt = xpool.tile([128, FREE], BF16)
            # 8 chunks of 2 psum tiles each, reads split between the SP
            # HWDGE ring and the GPSIMD SWDGE ring (two concurrent read
            # streams per SDMA engine; 16 KiB descriptors measured
            # fastest in the engines' queue rotation).  Chunk 1 goes
            # entirely on the SP ring (the SWDGE Q7 takes ~12 us to
            # boot) and chunk 2 entirely on SWDGE to rebalance.
            CH = 8192                      # elems per chunk per partition
            for k in range(FREE // CH):
                lo, hi = k * CH, (k + 1) * CH
                ring0 = nc.gpsimd if k == 1 else nc.sync
                ring1 = nc.sync if k == 0 else nc.gpsimd
                ring0.dma_start(
                    out=xt[0:C, lo:hi], in_=x[0, :, :, lo:hi])
                ring1.dma_start(
                    out=xt[C:2 * C, lo:hi], in_=x[1, :, :, lo:hi])
            xv = xt.rearrange(
                "p (bi hh pt di il j) -> p bi hh pt di il j",
                bi=B_LOCAL, hh=2, pt=4, di=2, j=WO,
            )
            for bi in range(B_LOCAL):
                for hh in range(2):
                    for pp in range(2):   # pt pair -> one 1 MiB store
                        zt = zpool.tile([COUT, 4096], BF16)
                        for sub in range(2):
                            pt = 2 * pp + sub
                            ps = ppool.tile([COUT, 2048], F32)
                            # output rows 64hh + 16pt + (4gg + 0..3)
                            for di in range(2):
                                for gg in range(4):
                                    nc.tensor.matmul(
                                        ps[:, gg * 512:gg * 512 + 512],
                                        lhsT=w_sb[di][:],
                                        rhs=xv[:, bi, hh, pt, di,
                                               4 * gg:4 * gg + 4, :],
                                        start=(di == 0),
                                        stop=(di == 1),
                                    )
                            zh = zt[:, sub * 2048:(sub + 1) * 2048]
                            # bias + ReLU; alternate engines so the PSUM
                            # drain is never single-engine-bound
                            if sub == 0:
                                nc.vector.tensor_scalar(
                                    zh, ps[:], bias_sb[:, 0:1], 0.0,
                                    mybir.AluOpType.add, mybir.AluOpType.max,
                                )
                            else:
                                nc.scalar.activation(
                                    zh, ps[:], relu, bias=bias_sb[:, 0:1],
                                )
                        # contiguous 1 MiB store, 8 KiB descriptors,
                        # DRAM outer dim o=128 -> sprays all engines.
                        # ACT ring: small, preempts reads briefly.
                        nc.scalar.dma_start(
                            out=z[bi, 2 * hh + pp], in_=zt[:])
    if run_bacc_compile:
        nc.compile()
    return nc


_NC_CACHE = {}


def _get_nc():
    if "nc" not in _NC_CACHE:
        _NC_CACHE["nc"] = build_nc()
    return _NC_CACHE["nc"]


def _tile_x(x):
    """[B, C, H, W] f32 -> [N_CORES, 2(dj), G, CG, FREE+PAD_E] bf16.

    Partition p = dj*64 + c (c = 4g+cg); per-partition free layout
    [bi, hh, pt, di, ilp, j] where input row = 128hh+32pt+2ilp+di,
    input col = 2j+dj.  2 KiB pad per channel block keeps the DMA AP
    dims from merging (preserves the 16x4 descriptor grouping).
    """
    xb = x.astype(ml_dtypes.bfloat16)
    # [core, bi, g, cg, hh, pt, ilp, di, j, dj]
    t = xb.reshape(N_CORES, B_LOCAL, G, CG, 2, 4, 16, 2, WO, 2)
    # -> [core, dj, g, cg, bi, hh, pt, di, ilp, j]
    t = np.ascontiguousarray(t.transpose(0, 9, 2, 3, 1, 4, 5, 7, 6, 8))
    t = t.reshape(N_CORES, 2, G, CG, FREE)
    out = np.zeros((N_CORES, 2, G, CG, FREE + PAD_E),
                   dtype=ml_dtypes.bfloat16)
    out[:, :, :, :, 0:FREE] = t
    return out


def kernel(x, W, b, gamma, beta, mean, var, _trace=False):
    x = np.asarray(x, dtype=np.float32)
    lhsT, bias_col = _fold_weights(
        np.asarray(W), np.asarray(b), np.asarray(gamma),
        np.asarray(beta), np.asarray(mean), np.asarray(var),
    )
    x_t = _tile_x(x)

    nc = _get_nc()
    in_maps = []
    for core in range(N_CORES):
        in_maps.append({"x": x_t[core], "w_lhsT": lhsT, "bias": bias_col})

    res = run_bass_kernel_spmd(
        nc, in_maps, list(range(N_CORES)), trace=_trace
    )
    # z tiles [b_local, hp, o, 32*WO] bf16 -> [B, COUT, HO, WO] f32
    zt = np.concatenate(
        [np.asarray(res.results[i]["z"]) for i in range(N_CORES)], axis=0
    ).astype(np.float32)
    out = np.ascontiguousarray(
        zt.reshape(B, HB // 2, COUT, 32, WO).transpose(0, 2, 1, 3, 4)
    ).reshape(B, COUT, HO, WO)
    if _trace:
        return out, res
    return out


# revision 22
# speedup vs baseline: 1.1242x; 1.0499x over previous
"""Haar-DWT downsampling + 1x1 conv + BN + ReLU fused Trainium2 kernel.

Math: the Haar DWT (J=1) followed by a 1x1 conv over the 4C subband
channels, inference BN, and ReLU folds into a 2x2/stride-2 conv:

    z[o, i, j] = relu( sum_{c,di,dj} Weff[o, c, di, dj] * x[c, 2i+di, 2j+dj]
                       + bias_total[o] )

with Weff/bias_total computed on the host from (W, b, gamma, beta, mean,
var).

Device mapping: SBUF partition = (dj, c) so each matmul contracts
K = 128 = (dj, c) in one shot with a full 128x128 weight (enables the
PE fast-weight-load path; K=64 variants measured ~3x slower per row).
Only 2 weight matrices (one per di tap); each 16-output-row PSUM tile
takes 8 matmuls (2 di x 4 banks).

Precision: x and weights bf16 on host, z produced in bf16 (PSUM
accumulation f32) — halves HBM traffic, which is the roofline.
Measured rel err ~3.5e-3 vs the 2e-2 gate.

DMA plan (SDMA read throughput measures ~13 B/ns per engine regardless
of descriptor size >= 8 KiB, so scheduling, not descriptor shaping, is
what's left):
- ALL x loads issue on the SP HWDGE ring in FIFO order; the two rings
  do NOT share engines fairly (ACT-ring work starves SP-ring work), so
  reads are never split across rings.  Weights/bias go on the ACT ring
  so they don't delay the first x chunk.
- One x tile holds the whole per-core input (partition (dj,c), free
  [bi, hh, pt, di, ilp, j] = 128 KiB/partition).  Loads are chunked
  (bi0,hh0,pt0) -> rest-of-bi0 -> bi1, per dj half, so the first
  matmuls start ~12 us in while the rest streams.
- x DRAM: [dj, g(16), cg(4), free (+2 KiB pad)]: DRAM AP
  [(g,16),(cg,4),(contig)] -> 16 outer entries, one per SDMA engine.
- z stores (bf16 tiles [hb, o, 2048], contiguous 512 KiB, outer dim
  o=128) issue on the GPSIMD SWDGE ring so they never block loads;
  host un-tiles.
- bias+ReLU alternates DVE / ACT so PSUM drain is two-engine wide.

Sharding: pure data-parallel over batch. B=16 -> 2 images per core on
8 cores.
"""

import numpy as np
import ml_dtypes

import concourse.bacc as bacc
import concourse.mybir as mybir
from concourse.tile import TileContext
from concourse.bass_utils import run_bass_kernel_spmd

BN_EPS = 1e-5

# Problem shape (hardcoded per harness contract)
B, C, H, W_IMG = 16, 64, 256, 256
COUT = 128
N_CORES = 8
B_LOCAL = B // N_CORES          # 2 images per core
HO, WO = H // 2, W_IMG // 2     # 128 x 128 output image

G = 16                          # channel groups (1 per SDMA engine)
CG = C // G                     # 4 channels per group
FREE1 = 2 * 4 * 2 * 16 * WO     # per-image free = [hh,pt,di,ilp,j] = 32768
FREE = B_LOCAL * FREE1          # per-partition free = 65536 elems (128 KiB)
PAD_E = 1024                    # 2 KiB bf16 pad per channel block
HB = 8                          # 16-output-row store blocks per image

F32 = mybir.dt.float32
BF16 = mybir.dt.bfloat16


def _fold_weights(W, b, gamma, beta, mean, var):
    """Fold DWT + conv + BN into per-di lhsT weights [2, 128(K=(dj,c)),
    128(M=o)] (bf16) and a per-channel f32 bias [COUT, 1]."""
    W = W.astype(np.float64)
    Wll, Wlh, Whl, Whh = W[:, :C], W[:, C:2 * C], W[:, 2 * C:3 * C], W[:, 3 * C:]
    s = (gamma.astype(np.float64) / np.sqrt(var.astype(np.float64) + BN_EPS))
    coef = {
        (0, 0): 0.5 * (Wll + Wlh + Whl + Whh),
        (0, 1): 0.5 * (Wll + Wlh - Whl - Whh),
        (1, 0): 0.5 * (Wll - Wlh + Whl - Whh),
        (1, 1): 0.5 * (Wll - Wlh - Whl + Whh),
    }
    bias_total = (b.astype(np.float64) * s + beta.astype(np.float64)
                  - mean.astype(np.float64) * s)
    lhsT = np.zeros((2, 128, COUT), dtype=np.float64)
    for di in range(2):
        for dj in range(2):
            wq = (coef[(di, dj)] * s[:, None]).T   # [c, o]
            lhsT[di, dj * C:(dj + 1) * C, :] = wq
    wcat = np.concatenate([lhsT[0], lhsT[1]], axis=1)   # [128, 2*COUT]
    return (np.ascontiguousarray(wcat).astype(ml_dtypes.bfloat16),
            bias_total.astype(np.float32).reshape(COUT, 1))


def build_nc(run_bacc_compile=True):
    nc = bacc.Bacc(None)
    x = nc.dram_tensor("x", [2, G, CG, FREE + PAD_E], BF16,
                       kind="ExternalInput")
    # both di weight matrices side by side -> one DMA, 512 B descriptors
    w_lhsT = nc.dram_tensor("w_lhsT", [128, 2 * COUT], BF16,
                            kind="ExternalInput")
    bias = nc.dram_tensor("bias", [COUT, 1], F32, kind="ExternalInput")
    # 32-output-row tiles: contiguous 1 MiB per store, 8 KiB descriptors
    z = nc.dram_tensor("z", [B_LOCAL, HB // 2, COUT, 32 * WO], BF16,
                       kind="ExternalOutput")

    relu = mybir.ActivationFunctionType.Relu

    with TileContext(nc) as tc:
        with (
            tc.tile_pool(name="consts", bufs=1) as cpool,
            tc.tile_pool(name="xin", bufs=1) as xpool,
            tc.tile_pool(name="psum", bufs=2, space="PSUM") as ppool,
            tc.tile_pool(name="zout", bufs=4) as zpool,
        ):
            # weights/bias on the ACT ring so they never delay x chunks
            wt = cpool.tile([128, 2 * COUT], BF16, name="w_sb")
            nc.scalar.dma_start(out=wt[:], in_=w_lhsT[:])
            w_sb = [wt[:, di * COUT:(di + 1) * COUT] for di in range(2)]
            bias_sb = cpool.tile([COUT, 1], F32)
            nc.scalar.dma_start(out=bias_sb[:], in_=bias[:])

            # whole per-core input in one tile: partition (dj, c), free
            # [bi, hh, pt, di, ilp, j] = 128 KiB bf16 per partition.
            # 8 chunks of 2 psum tiles each; dj=0 on the SP HWDGE ring,
            # dj=1 on the GPSIMD SWDGE ring — two concurrent read
            # streams per SDMA engine (the per-stream rate cap is ~13
            # B/ns; two streams overlap it).
            xt = xpool.tile([128, FREE], BF16)
            # 8 chunks of 2 psum tiles each, reads split between the SP
            # HWDGE ring and the GPSIMD SWDGE ring (two concurrent read
            # streams per SDMA engine; 16 KiB descriptors measured
            # fastest in the engines' queue rotation).  Chunk 1 goes
            # entirely on the SP ring (the SWDGE Q7 takes ~12 us to
            # boot) and chunk 2 entirely on SWDGE to rebalance.
            CH = 8192                      # elems per chunk per partition
            for k in range(FREE // CH):
                lo, hi = k * CH, (k + 1) * CH
                nc.sync.dma_start(
                    out=xt[0:C, lo:hi], in_=x[0, :, :, lo:hi])
                nc.gpsimd.dma_start(
                    out=xt[C:2 * C, lo:hi], in_=x[1, :, :, lo:hi])
            xv = xt.rearrange(
                "p (bi hh pt di il j) -> p bi hh pt di il j",
                bi=B_LOCAL, hh=2, pt=4, di=2, j=WO,
            )
            for bi in range(B_LOCAL):
                for hh in range(2):
                    for pp in range(2):   # pt pair -> one 1 MiB store
                        zt = zpool.tile([COUT, 4096], BF16)
                        for sub in range(2):
                            pt = 2 * pp + sub
                            ps = ppool.tile([COUT, 2048], F32)
                            # output rows 64hh + 16pt + (4gg + 0..3)
                            for di in range(2):
                                for gg in range(4):
                                    nc.tensor.matmul(
                                        ps[:, gg * 512:gg * 512 + 512],
                                        lhsT=w_sb[di],
                                        rhs=xv[:, bi, hh, pt, di,
                                               4 * gg:4 * gg + 4, :],
                                        start=(di == 0),
                                        stop=(di == 1),
                                    )
                            zh = zt[:, sub * 2048:(sub + 1) * 2048]
                            # bias + ReLU; alternate engines so the PSUM
                            # drain is never single-engine-bound
                            if sub == 0:
                                nc.vector.tensor_scalar(
                                    zh, ps[:], bias_sb[:, 0:1], 0.0,
                                    mybir.AluOpType.add, mybir.AluOpType.max,
                                )
                            else:
                                nc.scalar.activation(
                                    zh, ps[:], relu, bias=bias_sb[:, 0:1],
                                )
                        # contiguous 1 MiB store, 8 KiB descriptors,
                        # DRAM outer dim o=128 -> sprays all engines.
                        # ACT ring: small, preempts reads briefly.
                        nc.scalar.dma_start(
                            out=z[bi, 2 * hh + pp], in_=zt[:])
    if run_bacc_compile:
        nc.compile()
    return nc


_NC_CACHE = {}


def _get_nc():
    if "nc" not in _NC_CACHE:
        _NC_CACHE["nc"] = build_nc()
    return _NC_CACHE["nc"]


def _tile_x(x):
    """[B, C, H, W] f32 -> [N_CORES, 2(dj), G, CG, FREE+PAD_E] bf16.

    Partition p = dj*64 + c (c = 4g+cg); per-partition free layout
    [bi, hh, pt, di, ilp, j] where input row = 128hh+32pt+2ilp+di,
    input col = 2j+dj.  2 KiB pad per channel block keeps the DMA AP
    dims from merging (preserves the 16x4 descriptor grouping).
    """
    xb = x.astype(ml_dtypes.bfloat16)
    # [core, bi, g, cg, hh, pt, ilp, di, j, dj]
    t = xb.reshape(N_CORES, B_LOCAL, G, CG, 2, 4, 16, 2, WO, 2)
    # -> [core, dj, g, cg, bi, hh, pt, di, ilp, j]
    t = np.ascontiguousarray(t.transpose(0, 9, 2, 3, 1, 4, 5, 7, 6, 8))
    t = t.reshape(N_CORES, 2, G, CG, FREE)
    out = np.zeros((N_CORES, 2, G, CG, FREE + PAD_E),
                   dtype=ml_dtypes.bfloat16)
    out[:, :, :, :, 0:FREE] = t
    return out


def kernel(x, W, b, gamma, beta, mean, var, _trace=False):
    x = np.asarray(x, dtype=np.float32)
    lhsT, bias_col = _fold_weights(
        np.asarray(W), np.asarray(b), np.asarray(gamma),
        np.asarray(beta), np.asarray(mean), np.asarray(var),
    )
    x_t = _tile_x(x)

    nc = _get_nc()
    in_maps = []
    for core in range(N_CORES):
        in_maps.append({"x": x_t[core], "w_lhsT": lhsT, "bias": bias_col})

    res = run_bass_kernel_spmd(
        nc, in_maps, list(range(N_CORES)), trace=_trace
    )
    # z tiles [b_local, hp, o, 32*WO] bf16 -> [B, COUT, HO, WO] f32
    zt = np.concatenate(
        [np.asarray(res.results[i]["z"]) for i in range(N_CORES)], axis=0
    ).astype(np.float32)
    out = np.ascontiguousarray(
        zt.reshape(B, HB // 2, COUT, 32, WO).transpose(0, 2, 1, 3, 4)
    ).reshape(B, COUT, HO, WO)
    if _trace:
        return out, res
    return out


# revision 25
# speedup vs baseline: 1.1766x; 1.0466x over previous
"""Haar-DWT downsampling + 1x1 conv + BN + ReLU fused Trainium2 kernel.

Math: the Haar DWT (J=1) followed by a 1x1 conv over the 4C subband
channels, inference BN, and ReLU folds into a 2x2/stride-2 conv:

    z[o, i, j] = relu( sum_{c,di,dj} Weff[o, c, di, dj] * x[c, 2i+di, 2j+dj]
                       + bias_total[o] )

with Weff/bias_total computed on the host from (W, b, gamma, beta, mean,
var).

Device mapping: SBUF partition = (dj, c) so each matmul contracts
K = 128 = (dj, c) in one shot with a full 128x128 weight (enables the
PE fast-weight-load path; K=64 variants measured ~3x slower per row).
Only 2 weight matrices (one per di tap); each 16-output-row PSUM tile
takes 8 matmuls (2 di x 4 banks).

Precision: x and weights bf16 on host, z produced in bf16 (PSUM
accumulation f32) — halves HBM traffic, which is the roofline.
Measured rel err ~3.5e-3 vs the 2e-2 gate.

DMA plan (SDMA read throughput measures ~13 B/ns per engine regardless
of descriptor size >= 8 KiB, so scheduling, not descriptor shaping, is
what's left):
- ALL x loads issue on the SP HWDGE ring in FIFO order; the two rings
  do NOT share engines fairly (ACT-ring work starves SP-ring work), so
  reads are never split across rings.  Weights/bias go on the ACT ring
  so they don't delay the first x chunk.
- One x tile holds the whole per-core input (partition (dj,c), free
  [bi, hh, pt, di, ilp, j] = 128 KiB/partition).  Loads are chunked
  (bi0,hh0,pt0) -> rest-of-bi0 -> bi1, per dj half, so the first
  matmuls start ~12 us in while the rest streams.
- x DRAM: [dj, g(16), cg(4), free (+2 KiB pad)]: DRAM AP
  [(g,16),(cg,4),(contig)] -> 16 outer entries, one per SDMA engine.
- z stores (bf16 tiles [hb, o, 2048], contiguous 512 KiB, outer dim
  o=128) issue on the GPSIMD SWDGE ring so they never block loads;
  host un-tiles.
- bias+ReLU alternates DVE / ACT so PSUM drain is two-engine wide.

Sharding: pure data-parallel over batch. B=16 -> 2 images per core on
8 cores.
"""

import numpy as np
import ml_dtypes

import concourse.bacc as bacc
import concourse.mybir as mybir
from concourse.tile import TileContext
from concourse.bass_utils import run_bass_kernel_spmd

BN_EPS = 1e-5

# Problem shape (hardcoded per harness contract)
B, C, H, W_IMG = 16, 64, 256, 256
COUT = 128
N_CORES = 8
B_LOCAL = B // N_CORES          # 2 images per core
HO, WO = H // 2, W_IMG // 2     # 128 x 128 output image

G = 16                          # channel groups (1 per SDMA engine)
CG = C // G                     # 4 channels per group
FREE1 = 2 * 4 * 2 * 16 * WO     # per-image free = [hh,pt,di,ilp,j] = 32768
FREE = B_LOCAL * FREE1          # per-partition free = 65536 elems (128 KiB)
PAD_E = 1024                    # 2 KiB bf16 pad per channel block
HB = 8                          # 16-output-row store blocks per image

F32 = mybir.dt.float32
BF16 = mybir.dt.bfloat16


def _fold_weights(W, b, gamma, beta, mean, var):
    """Fold DWT + conv + BN into per-di lhsT weights [2, 128(K=(dj,c)),
    128(M=o)] (bf16) and a per-channel f32 bias [COUT, 1]."""
    W = W.astype(np.float64)
    Wll, Wlh, Whl, Whh = W[:, :C], W[:, C:2 * C], W[:, 2 * C:3 * C], W[:, 3 * C:]
    s = (gamma.astype(np.float64) / np.sqrt(var.astype(np.float64) + BN_EPS))
    coef = {
        (0, 0): 0.5 * (Wll + Wlh + Whl + Whh),
        (0, 1): 0.5 * (Wll + Wlh - Whl - Whh),
        (1, 0): 0.5 * (Wll - Wlh + Whl - Whh),
        (1, 1): 0.5 * (Wll - Wlh - Whl + Whh),
    }
    bias_total = (b.astype(np.float64) * s + beta.astype(np.float64)
                  - mean.astype(np.float64) * s)
    lhsT = np.zeros((2, 128, COUT), dtype=np.float64)
    for di in range(2):
        for dj in range(2):
            wq = (coef[(di, dj)] * s[:, None]).T   # [c, o]
            lhsT[di, dj * C:(dj + 1) * C, :] = wq
    wcat = np.concatenate([lhsT[0], lhsT[1]], axis=1)   # [128, 2*COUT]
    return (np.ascontiguousarray(wcat).astype(ml_dtypes.bfloat16),
            bias_total.astype(np.float32).reshape(COUT, 1))


def build_nc(run_bacc_compile=True):
    nc = bacc.Bacc(None)
    x = nc.dram_tensor("x", [2, G, CG, FREE + PAD_E], BF16,
                       kind="ExternalInput")
    # both di weight matrices side by side -> one DMA, 512 B descriptors
    w_lhsT = nc.dram_tensor("w_lhsT", [128, 2 * COUT], BF16,
                            kind="ExternalInput")
    bias = nc.dram_tensor("bias", [COUT, 1], F32, kind="ExternalInput")
    # 32-output-row tiles: contiguous 1 MiB per store, 8 KiB descriptors
    z = nc.dram_tensor("z", [B_LOCAL, HB // 2, COUT, 32 * WO], BF16,
                       kind="ExternalOutput")

    relu = mybir.ActivationFunctionType.Relu

    with TileContext(nc) as tc:
        with (
            tc.tile_pool(name="consts", bufs=1) as cpool,
            tc.tile_pool(name="xin", bufs=1) as xpool,
            tc.tile_pool(name="psum", bufs=2, space="PSUM") as ppool,
            tc.tile_pool(name="zout", bufs=6) as zpool,
        ):
            # weights/bias on the ACT ring so they never delay x chunks
            wt = cpool.tile([128, 2 * COUT], BF16, name="w_sb")
            nc.scalar.dma_start(out=wt[:], in_=w_lhsT[:])
            w_sb = [wt[:, di * COUT:(di + 1) * COUT] for di in range(2)]
            bias_sb = cpool.tile([COUT, 1], F32)
            nc.scalar.dma_start(out=bias_sb[:], in_=bias[:])

            # whole per-core input in one tile: partition (dj, c), free
            # [bi, hh, pt, di, ilp, j] = 128 KiB bf16 per partition.
            # 8 chunks of 2 psum tiles each; dj=0 on the SP HWDGE ring,
            # dj=1 on the GPSIMD SWDGE ring — two concurrent read
            # streams per SDMA engine (the per-stream rate cap is ~13
            # B/ns; two streams overlap it).
            xt = xpool.tile([128, FREE], BF16)
            # 8 chunks of 2 psum tiles each, reads split between the SP
            # HWDGE ring and the GPSIMD SWDGE ring (two concurrent read
            # streams per SDMA engine; 16 KiB descriptors measured
            # fastest in the engines' queue rotation).  Chunk 1 goes
            # entirely on the SP ring (the SWDGE Q7 takes ~12 us to
            # boot) and chunk 2 entirely on SWDGE to rebalance.
            # last chunk split in two so the final compute+store tail
            # starts as early as possible
            bounds = (0, 8192, 16384, 24576, 32768, 40960, 49152,
                      57344, 61440, FREE)
            for k in range(len(bounds) - 1):
                lo, hi = bounds[k], bounds[k + 1]
                nc.sync.dma_start(
                    out=xt[0:C, lo:hi], in_=x[0, :, :, lo:hi])
                nc.gpsimd.dma_start(
                    out=xt[C:2 * C, lo:hi], in_=x[1, :, :, lo:hi])
            xv = xt.rearrange(
                "p (bi hh pt di il j) -> p bi hh pt di il j",
                bi=B_LOCAL, hh=2, pt=4, di=2, j=WO,
            )
            for bi in range(B_LOCAL):
                for hh in range(2):
                    for pp in range(2):   # pt pair -> one 1 MiB store
                        zt = zpool.tile([COUT, 4096], BF16)
                        for sub in range(2):
                            pt = 2 * pp + sub
                            ps = ppool.tile([COUT, 2048], F32)
                            # output rows 64hh + 16pt + (4gg + 0..3)
                            for di in range(2):
                                for gg in range(4):
                                    nc.tensor.matmul(
                                        ps[:, gg * 512:gg * 512 + 512],
                                        lhsT=w_sb[di],
                                        rhs=xv[:, bi, hh, pt, di,
                                               4 * gg:4 * gg + 4, :],
                                        start=(di == 0),
                                        stop=(di == 1),
                                    )
                            zh = zt[:, sub * 2048:(sub + 1) * 2048]
                            # bias + ReLU; alternate engines so the PSUM
                            # drain is never single-engine-bound
                            if sub == 0:
                                nc.vector.tensor_scalar(
                                    zh, ps[:], bias_sb[:, 0:1], 0.0,
                                    mybir.AluOpType.add, mybir.AluOpType.max,
                                )
                            else:
                                nc.scalar.activation(
                                    zh, ps[:], relu, bias=bias_sb[:, 0:1],
                                )
                        # contiguous 1 MiB store, 8 KiB descriptors,
                        # DRAM outer dim o=128 -> sprays all engines.
                        # Alternate ACT / SWDGE rings so the final
                        # stores drain on two queues, not one.
                        sidx = bi * 4 + 2 * hh + pp
                        ring = nc.scalar if sidx % 2 == 0 else nc.gpsimd
                        ring.dma_start(
                            out=z[bi, 2 * hh + pp], in_=zt[:])
    if run_bacc_compile:
        nc.compile()
    return nc


_NC_CACHE = {}


def _get_nc():
    if "nc" not in _NC_CACHE:
        _NC_CACHE["nc"] = build_nc()
    return _NC_CACHE["nc"]


def _tile_x(x):
    """[B, C, H, W] f32 -> [N_CORES, 2(dj), G, CG, FREE+PAD_E] bf16.

    Partition p = dj*64 + c (c = 4g+cg); per-partition free layout
    [bi, hh, pt, di, ilp, j] where input row = 128hh+32pt+2ilp+di,
    input col = 2j+dj.  2 KiB pad per channel block keeps the DMA AP
    dims from merging (preserves the 16x4 descriptor grouping).
    """
    xb = x.astype(ml_dtypes.bfloat16)
    # [core, bi, g, cg, hh, pt, ilp, di, j, dj]
    t = xb.reshape(N_CORES, B_LOCAL, G, CG, 2, 4, 16, 2, WO, 2)
    # -> [core, dj, g, cg, bi, hh, pt, di, ilp, j]
    t = np.ascontiguousarray(t.transpose(0, 9, 2, 3, 1, 4, 5, 7, 6, 8))
    t = t.reshape(N_CORES, 2, G, CG, FREE)
    out = np.zeros((N_CORES, 2, G, CG, FREE + PAD_E),
                   dtype=ml_dtypes.bfloat16)
    out[:, :, :, :, 0:FREE] = t
    return out


def kernel(x, W, b, gamma, beta, mean, var, _trace=False):
    x = np.asarray(x, dtype=np.float32)
    lhsT, bias_col = _fold_weights(
        np.asarray(W), np.asarray(b), np.asarray(gamma),
        np.asarray(beta), np.asarray(mean), np.asarray(var),
    )
    x_t = _tile_x(x)

    nc = _get_nc()
    in_maps = []
    for core in range(N_CORES):
        in_maps.append({"x": x_t[core], "w_lhsT": lhsT, "bias": bias_col})

    res = run_bass_kernel_spmd(
        nc, in_maps, list(range(N_CORES)), trace=_trace
    )
    # z tiles [b_local, hp, o, 32*WO] bf16 -> [B, COUT, HO, WO] f32
    zt = np.concatenate(
        [np.asarray(res.results[i]["z"]) for i in range(N_CORES)], axis=0
    ).astype(np.float32)
    out = np.ascontiguousarray(
        zt.reshape(B, HB // 2, COUT, 32, WO).transpose(0, 2, 1, 3, 4)
    ).reshape(B, COUT, HO, WO)
    if _trace:
        return out, res
    return out
